# revision 11
# baseline (speedup 1.0000x reference)
"""Trainium2 Bass kernel for nn_DictionaryLearningTokenized (vq_codebook).

Batched OMP (K=4) over 131072 signals of dim 64 against a 256-atom
dictionary, mu-law quantization of the coefficients, reconstruction and
VQ-VAE loss.  Data-parallel over 8 NeuronCores: each core handles a
[4, 64, 64, 64] shard of z_e (16384 signals); the small dictionary is
replicated.  The final loss mean is reduced on the host.

Device algorithm (per 1024-signal block, QR/MGS form of batch OMP —
numerically validated against the jax reference in float32):
  h      = X @ Dn                      (PE, residual correlations, PSUM)
  k-th iteration:
    habs = |h|                         (ACT)
    vmax = rowmax(habs)                (DVE segmented reduce)
    E    = (habs == vmax)              (one-hot of the argmax; DVE/Pool)
    E^T  via PE transposes; atom A_k = E @ DnT via PE one-hot matmul
    w_j  = <q_j, A_k>, hb = <X, A_k>   (64-wide dots, DVE/Pool)
    val  = hb - sum_j y_j w_j ; c_k = sqrt(clip(1 - sum w_j^2))
    y_k  = val / c_k ; q_k = (A_k - sum_j w_j q_j) / c_k
    h   += (-y_k q_k) @ Dn             (PE matmul accumulate into PSUM)
  backsolve L^T x = y, mu-law quantize x, recon = sum_k xq_k A_k,
  z_q = X + (recon - X), transpose back and DMA out.
"""

import math
from contextlib import ExitStack

import numpy as np

# ---- problem constants (hardcoded per spec) ----
N_CORES = 8
B_FULL, CD, Hh, Ww = 32, 64, 64, 64
HW = Hh * Ww                     # 4096
B_PER_CORE = B_FULL // N_CORES   # 4
NA = 256                         # atoms
K = 4                            # sparsity
TC = 4                           # chunks (of 128 signals) per block
BLK = TC * 128                   # 1024 signals per block
N_BLOCKS = B_PER_CORE * HW // BLK  # 16 blocks per core

MU = 50.0
CMAX = 3.0
NBINS = 16
EPS = 1e-10
COMMIT = 0.25
LOG1P_MU = float(np.log1p(50.0))          # float64, like reference
INV_LOG1P_MU = 1.0 / LOG1P_MU             # float64; f32-cast at use sites
F32 = None  # set lazily (mybir.dt.float32)

_NC_CACHE = {}


def _bcast_last(ap, n):
    """Append an innermost [step=0, count=n] dim to an AP (free-dim bcast)."""
    import concourse.bass as bass
    return bass.AP(tensor=ap.tensor, offset=ap.offset,
                   ap=[*[list(d) for d in ap.ap], [0, n]])


def build_nc(n_blocks=N_BLOCKS):
    import concourse.bass as bass
    import concourse.mybir as mybir
    import concourse.tile as tile
    from concourse import bacc
    from concourse.masks import make_identity

    f32 = mybir.dt.float32
    bf16 = mybir.dt.bfloat16
    ALU = mybir.AluOpType
    ACTF = mybir.ActivationFunctionType

    nc = bacc.Bacc(trn_type="TRN2", name="omp_vq")
    ze = nc.dram_tensor("ze", [B_PER_CORE, CD, HW], f32, kind="ExternalInput")
    dn = nc.dram_tensor("dn", [CD, NA], f32, kind="ExternalInput")
    dnt = nc.dram_tensor("dnt", [2, 128, CD], f32, kind="ExternalInput")
    zq = nc.dram_tensor("zq", [B_PER_CORE, CD, HW], f32, kind="ExternalOutput")

    dve = nc.vector
    act = nc.scalar
    pool_e = nc.gpsimd
    pe = nc.tensor

    def ts1(eng, out, in0, s1, op0):
        eng.tensor_scalar(out=out, in0=in0, scalar1=s1, scalar2=None, op0=op0)

    def ts2(eng, out, in0, s1, op0, s2, op1):
        eng.tensor_scalar(out=out, in0=in0, scalar1=s1, scalar2=s2,
                          op0=op0, op1=op1)

    with ExitStack() as ctx:
        tc = ctx.enter_context(tile.TileContext(nc))
        singles = ctx.enter_context(tc.tile_pool(name="singles", bufs=1))
        work = ctx.enter_context(tc.tile_pool(name="work", bufs=2))
        persist = ctx.enter_context(tc.tile_pool(name="persist", bufs=2))
        small = ctx.enter_context(tc.tile_pool(name="small", bufs=2))
        hpool = ctx.enter_context(tc.tile_pool(name="hps", bufs=2, space="PSUM"))
        trpool = ctx.enter_context(tc.tile_pool(name="trps", bufs=2, space="PSUM"))
        mmpool = ctx.enter_context(tc.tile_pool(name="mmps", bufs=2, space="PSUM"))

        # --- constants ---
        ident = singles.tile([128, 128], f32)
        make_identity(nc, ident)
        identb = singles.tile([128, 128], bf16)
        make_identity(nc, identb)
        dn_sb = singles.tile([CD, NA], f32)
        nc.sync.dma_start(out=dn_sb, in_=dn[:, :])
        dnt_sb = singles.tile([128, 2, CD], f32)
        for h in range(2):
            nc.sync.dma_start(out=dnt_sb[:, h, :], in_=dnt[h])

        # PE may carry at most one sem wait per instruction; consume each
        # constant's producer once so later matmuls never wait on them.
        warm = trpool.tile([128, 512], f32, tag="tr")
        pe.matmul(warm[0:1, 0:1], lhsT=dn_sb[:, 0:1], rhs=dn_sb[:, 0:1],
                  start=True, stop=True, skip_group_check=True)
        pe.matmul(warm[0:1, 1:2], lhsT=dnt_sb[:, 0, 0:1], rhs=dnt_sb[:, 0, 0:1],
                  start=True, stop=True, skip_group_check=True)
        pe.matmul(warm[0:1, 2:3], lhsT=dnt_sb[:, 1, 0:1], rhs=dnt_sb[:, 1, 0:1],
                  start=True, stop=True, skip_group_check=True)
        pe.matmul(warm[0:1, 3:4], lhsT=ident[:, 0:1], rhs=ident[:, 0:1],
                  start=True, stop=True, skip_group_check=True)

        for bi in range(n_blocks):
            b = bi // (HW // BLK)
            hw0 = (bi % (HW // BLK)) * BLK

            # ---------- setup: load X^T, h = X @ Dn, X signal-major ----------
            xt = work.tile([CD, BLK], f32, tag="xt")          # [64, 8*128]
            nc.sync.dma_start(out=xt, in_=ze[b, :, hw0:hw0 + BLK])

            # Chunks c and c+1 share a 2KB PSUM bank.  start=True marks the
            # WHOLE bank pending-zero, so only the even (bank-first) chunk may
            # use it; the odd chunk overwrites its still-pending half with
            # start=False.  Later psi-matmuls then accumulate (bits set).
            h_ps = hpool.tile([128, TC, NA], f32, tag="h")    # 4 banks
            for c in range(TC):
                pe.matmul(h_ps[:, c, :], lhsT=xt[:, c * 128:(c + 1) * 128],
                          rhs=dn_sb, start=(c % 2 == 0), stop=(c % 2 == 1),
                          skip_group_check=True)

            xs_ps = mmpool.tile([128, TC * CD], f32, tag="mm")  # 1 bank
            for c in range(TC):
                pe.transpose(xs_ps[:, c * CD:(c + 1) * CD],
                             xt[:, c * 128:(c + 1) * 128], ident[0:CD, 0:CD])
            x_sb = persist.tile([128, TC, CD], f32, tag="xsb")
            act.copy(x_sb, xs_ps.rearrange("p (t c) -> p t c", t=TC))

            # per-block per-signal state
            L_all = persist.tile([128, K, K, TC], f32, tag="Lall")
            y_all = persist.tile([128, K, TC], f32, tag="yall")
            a_all = persist.tile([128, K, TC, CD], f32, tag="Aall")
            q_sb = persist.tile([128, K - 1, TC, CD], f32, tag="qsb")

            for k in range(K):
                # ---------- selection ----------
                habs = work.tile([128, TC, NA], f32, tag="habs")
                act.activation(habs, h_ps, ACTF.Abs)
                vmax = small.tile([128, TC], f32, tag="vmax")
                dve.tensor_reduce(out=vmax, in_=habs, axis=mybir.AxisListType.X,
                                  op=ALU.max)
                e_t = work.tile([128, TC, NA], bf16, tag="E")
                for c in range(TC):
                    ts1(dve, e_t[:, c, :], habs[:, c, :], vmax[:, c:c + 1],
                        ALU.is_equal)

                # ---------- E^T via PE transposes (bf16, waves of 4) ----------
                et_sb = work.tile([128, 2 * TC, 128], f32, tag="etsb")
                for w in range(2 * TC // 4):
                    trb = trpool.tile([128, 512], bf16, tag="tr")
                    for i in range(4):
                        c = 2 * w + i // 2
                        hh = i % 2
                        pe.transpose(trb[:, i * 128:(i + 1) * 128],
                                     e_t[:, c, hh * 128:(hh + 1) * 128], identb)
                    act.copy(et_sb[:, 4 * w:4 * w + 4, :],
                             trb.rearrange("p (i q) -> p i q", i=4))

                # ---------- atom gather: A_k = E @ DnT ----------
                a_ps = mmpool.tile([128, TC * CD], f32, tag="mm")
                for c in range(TC):
                    for hh in range(2):
                        pe.matmul(a_ps[:, c * CD:(c + 1) * CD],
                                  lhsT=et_sb[:, 2 * c + hh, :],
                                  rhs=dnt_sb[:, hh, :],
                                  start=(hh == 0), stop=(hh == 1))
                a_k = a_all[:, k, :, :]
                act.copy(a_k, a_ps.rearrange("p (t c) -> p t c", t=TC))

                # ---------- dots: w_j = <q_j, A_k>, hb = <X, A_k> ----------
                hb = small.tile([128, TC], f32, tag="hb")
                for j in range(k + 1):
                    dotbuf = work.tile([128, TC, CD], f32, tag="dot")
                    opnd = q_sb[:, j, :, :] if j < k else x_sb
                    outt = L_all[:, k, j, :] if j < k else hb
                    pool_e.tensor_tensor(out=dotbuf, in0=opnd, in1=a_k, op=ALU.mult)
                    dve.tensor_reduce(out=outt, in_=dotbuf,
                                      axis=mybir.AxisListType.X, op=ALU.add)

                # ---------- Cholesky scalars ----------
                if k == 0:
                    dve.memset(L_all[:, 0, 0, :], 1.0)
                    dve.tensor_copy(out=y_all[:, 0, :], in_=hb)
                else:
                    wsq = small.tile([128, TC], f32, tag="wsq")
                    yw = small.tile([128, TC], f32, tag="yw")
                    tmp8 = small.tile([128, TC], f32, tag="tmp8")
                    for j in range(k):
                        wj = L_all[:, k, j, :]
                        if j == 0:
                            dve.tensor_tensor(out=wsq, in0=wj, in1=wj, op=ALU.mult)
                            dve.tensor_tensor(out=yw, in0=y_all[:, 0, :], in1=wj,
                                              op=ALU.mult)
                        else:
                            dve.tensor_tensor(out=tmp8, in0=wj, in1=wj, op=ALU.mult)
                            dve.tensor_tensor(out=wsq, in0=wsq, in1=tmp8, op=ALU.add)
                            dve.tensor_tensor(out=tmp8, in0=y_all[:, j, :], in1=wj,
                                              op=ALU.mult)
                            dve.tensor_tensor(out=yw, in0=yw, in1=tmp8, op=ALU.add)
                    val8 = small.tile([128, TC], f32, tag="val8")
                    dve.tensor_tensor(out=val8, in0=hb, in1=yw, op=ALU.subtract)
                    onem = small.tile([128, TC], f32, tag="onem")
                    ts2(dve, onem, wsq, -1.0, ALU.mult, 1.0, ALU.add)
                    ts1(dve, onem, onem, 1e-12, ALU.max)
                    ck = L_all[:, k, k, :]
                    act.sqrt(ck, onem)
                    rcp8 = small.tile([128, TC], f32, tag="rcp8")
                    dve.reciprocal(out=rcp8, in_=ck)
                    dve.tensor_tensor(out=y_all[:, k, :], in0=val8, in1=rcp8,
                                      op=ALU.mult)

                # ---------- q_k and h update (not needed at k == K-1) ----------
                if k < K - 1:
                    qk = q_sb[:, k, :, :]
                    if k == 0:
                        # c_0 == 1 exactly; q_0 = A_0
                        dve.tensor_copy(out=qk, in_=a_k)
                    else:
                        accq = work.tile([128, TC, CD], f32, tag="accq")
                        src = a_k
                        for j in range(k):
                            tmpq = work.tile([128, TC, CD], f32, tag="tmpq")
                            pool_e.tensor_tensor(
                                out=tmpq, in0=q_sb[:, j, :, :],
                                in1=_bcast_last(L_all[:, k, j, :], CD), op=ALU.mult)
                            dve.tensor_tensor(out=accq, in0=src, in1=tmpq,
                                              op=ALU.subtract)
                            src = accq
                        dve.tensor_tensor(out=qk, in0=accq,
                                          in1=_bcast_last(rcp8, CD),
                                          op=ALU.mult)
                    # qt = q_k * (-y_k)
                    yneg = small.tile([128, TC], f32, tag="yneg")
                    ts1(dve, yneg, y_all[:, k, :], -1.0, ALU.mult)
                    qt = work.tile([128, TC, CD], f32, tag="qt")
                    dve.tensor_tensor(out=qt, in0=qk, in1=_bcast_last(yneg, CD),
                                      op=ALU.mult)
                    # transpose qt chunks -> [64, 128] and accumulate into h
                    qtt_sb = work.tile([CD, TC * 128], f32, tag="qtt")
                    for w in range(max(1, TC // 4)):
                        tr = trpool.tile([128, 512], f32, tag="tr")
                        for i in range(min(4, TC)):
                            c = 4 * w + i
                            pe.transpose(tr[0:CD, i * 128:(i + 1) * 128],
                                         qt[:, c, :], ident)
                        act.copy(qtt_sb[:, w * 512:(w + 1) * 512], tr[0:CD, :])
                    for c in range(TC):
                        pe.matmul(h_ps[:, c, :],
                                  lhsT=qtt_sb[:, c * 128:(c + 1) * 128],
                                  rhs=dn_sb, start=False, stop=True,
                                  skip_group_check=True)

            # ---------- backsolve L^T x = y ----------
            xsel = small.tile([128, TC, K], f32, tag="xsel")
            tmpb = small.tile([128, TC], f32, tag="tmpb")
            accb = small.tile([128, TC], f32, tag="accb")
            for m in range(K - 1, -1, -1):
                src = y_all[:, m, :]
                for j in range(m + 1, K):
                    dve.tensor_tensor(out=tmpb, in0=L_all[:, j, m, :],
                                      in1=xsel[:, :, j], op=ALU.mult)
                    dve.tensor_tensor(out=accb, in0=src, in1=tmpb, op=ALU.subtract)
                    src = accb
                rcpb = small.tile([128, TC], f32, tag="rcpb")
                dve.reciprocal(out=rcpb, in_=L_all[:, m, m, :])
                dve.tensor_tensor(out=xsel[:, :, m], in0=src,
                                  in1=rcpb, op=ALU.mult)

            # ---------- mu-law quantize (on [128, TC*K]) ----------
            xf = xsel.rearrange("p t k -> p (t k)")
            deq = small.tile([128, TC, K], f32, tag="deq")
            df = deq.rearrange("p t k -> p (t k)")
            t1 = small.tile([128, TC * K], f32, tag="qt1")
            t2 = small.tile([128, TC * K], f32, tag="qt2")
            t3 = small.tile([128, TC * K], f32, tag="qt3")
            ts2(dve, t1, xf, -CMAX, ALU.max, CMAX, ALU.min)    # clip
            ts1(dve, t1, t1, 1.0 / CMAX, ALU.mult)             # c = clip/3
            act.activation(t2, t1, ACTF.Abs)                   # |c|
            act.sign(t3, t1)                                   # sign(c)
            act.activation(t2, t2, ACTF.Ln, bias=1.0, scale=MU)  # ln(1+50|c|)
            dve.tensor_tensor(out=t2, in0=t3, in1=t2, op=ALU.mult)
            ts1(dve, t2, t2, INV_LOG1P_MU, ALU.mult)           # encoded
            ts2(dve, t2, t2, 1.0, ALU.add, (NBINS - 1) / 2.0, ALU.mult)
            ts1(dve, t2, t2, 8388608.0, ALU.add)               # round (RNE)
            ts1(dve, t2, t2, 8388608.0, ALU.subtract)
            ts2(dve, t2, t2, 0.0, ALU.max, float(NBINS - 1), ALU.min)  # bin
            ts2(dve, t2, t2, 2.0 / (NBINS - 1), ALU.mult, -1.0, ALU.add)  # z
            act.activation(t1, t2, ACTF.Abs)                   # |z|
            act.sign(t3, t2)                                   # sign(z)
            ts1(dve, t1, t1, LOG1P_MU, ALU.mult)
            act.activation(t1, t1, ACTF.Exp)
            ts1(dve, t1, t1, -1.0, ALU.add)                    # expm1
            ts1(dve, t1, t1, 1.0 / MU, ALU.mult)
            dve.tensor_tensor(out=t1, in0=t3, in1=t1, op=ALU.mult)
            ts1(dve, df, t1, CMAX, ALU.mult)                   # dequantized

            # ---------- recon, d, z_q ----------
            racc = work.tile([128, TC, CD], f32, tag="racc")
            rtmp = work.tile([128, TC, CD], f32, tag="rtmp")
            pool_e.tensor_tensor(out=racc, in0=a_all[:, 0, :, :],
                                 in1=_bcast_last(deq[:, :, 0], CD), op=ALU.mult)
            for m in range(1, K):
                pool_e.tensor_tensor(out=rtmp, in0=a_all[:, m, :, :],
                                     in1=_bcast_last(deq[:, :, m], CD), op=ALU.mult)
                dve.tensor_tensor(out=racc, in0=racc, in1=rtmp, op=ALU.add)
            dsb = work.tile([128, TC, CD], f32, tag="dsb")
            dve.tensor_tensor(out=dsb, in0=racc, in1=x_sb, op=ALU.subtract)
            zq_sb = work.tile([128, TC, CD], f32, tag="zqsb")
            pool_e.tensor_tensor(out=zq_sb, in0=x_sb, in1=dsb, op=ALU.add)

            # ---------- transpose z_q back and store ----------
            zqt_sb = work.tile([CD, BLK], f32, tag="zqt")
            for w in range(max(1, TC // 4)):
                tr = trpool.tile([128, 512], f32, tag="tr")
                for i in range(min(4, TC)):
                    c = 4 * w + i
                    pe.transpose(tr[0:CD, i * 128:(i + 1) * 128],
                                 zq_sb[:, c, :], ident)
                act.copy(zqt_sb[:, w * 512:(w + 1) * 512], tr[0:CD, :])
            nc.sync.dma_start(out=zq[b, :, hw0:hw0 + BLK], in_=zqt_sb)

    nc.compile()
    return nc


def _host_normalize(dictionary):
    """Normalize dictionary columns, matching reference numerics (f32)."""
    try:
        import jax
        cpu = jax.devices("cpu")[0]
        import jax.numpy as jnp
        with jax.default_device(cpu):
            d = jnp.asarray(dictionary, dtype=jnp.float32)
            nrm = jnp.linalg.norm(d, axis=0, keepdims=True)
            dn = d / jnp.maximum(nrm, EPS)
            return np.asarray(dn, dtype=np.float32)
    except Exception:
        d = dictionary.astype(np.float32)
        nrm = np.sqrt(np.sum(d * d, axis=0, keepdims=True, dtype=np.float32))
        return (d / np.maximum(nrm, np.float32(EPS))).astype(np.float32)


def _host_loss(z_q, z_e):
    """loss = q_latent + 0.25 * e_latent, replicating reference fp32 mean."""
    try:
        import jax
        cpu = jax.devices("cpu")[0]
        import jax.numpy as jnp
        with jax.default_device(cpu):
            d = jnp.asarray(z_q) - jnp.asarray(z_e)
            m = jnp.mean(d * d)
            loss = m + np.float32(COMMIT) * m
            return np.asarray(loss, dtype=np.float32)
    except Exception:
        d = (z_q - z_e).astype(np.float32)
        m = np.float32(np.mean(d.astype(np.float64) ** 2))
        return np.float32(m + np.float32(COMMIT) * m)


def get_nc():
    if "nc" not in _NC_CACHE:
        _NC_CACHE["nc"] = build_nc()
    return _NC_CACHE["nc"]


def _run(z_e, dictionary, **run_kwargs):
    from concourse.bass_utils import run_bass_kernel_spmd

    z_e = np.ascontiguousarray(np.asarray(z_e, dtype=np.float32))
    dictionary = np.ascontiguousarray(np.asarray(dictionary, dtype=np.float32))

    dn = _host_normalize(dictionary)                  # [64, 256]
    dnt = np.ascontiguousarray(dn.T).reshape(2, 128, CD).copy()

    ze_flat = z_e.reshape(B_FULL, CD, HW)
    in_maps = []
    for c in range(N_CORES):
        shard = np.ascontiguousarray(ze_flat[c * B_PER_CORE:(c + 1) * B_PER_CORE])
        in_maps.append({"ze": shard, "dn": dn, "dnt": dnt})

    nc = get_nc()
    res = run_bass_kernel_spmd(nc, in_maps, core_ids=list(range(N_CORES)),
                               **run_kwargs)
    zq_full = np.concatenate([r["zq"] for r in res.results], axis=0)
    zq_full = zq_full.reshape(B_FULL, CD, Hh, Ww)

    loss = _host_loss(zq_full, z_e)
    return zq_full, loss, res


def kernel(z_e, dictionary):
    zq_full, loss, _ = _run(z_e, dictionary)
    return zq_full, loss


# revision 12
# speedup vs baseline: 1.2705x; 1.2705x over previous
"""Trainium2 Bass kernel for nn_DictionaryLearningTokenized (vq_codebook).

Batched OMP (K=4) over 131072 signals of dim 64 against a 256-atom
dictionary, mu-law quantization of the coefficients, reconstruction and
VQ-VAE loss.  Data-parallel over 8 NeuronCores: each core handles a
[4, 64, 64, 64] shard of z_e (16384 signals); the small dictionary is
replicated.  The final loss mean is reduced on the host.

Device algorithm (per 1024-signal block, QR/MGS form of batch OMP —
numerically validated against the jax reference in float32):
  h      = X @ Dn                      (PE, residual correlations, PSUM)
  k-th iteration:
    habs = |h|                         (ACT)
    vmax = rowmax(habs)                (DVE segmented reduce)
    E    = (habs == vmax)              (one-hot of the argmax; DVE/Pool)
    E^T  via PE transposes; atom A_k = E @ DnT via PE one-hot matmul
    w_j  = <q_j, A_k>, hb = <X, A_k>   (64-wide dots, DVE/Pool)
    val  = hb - sum_j y_j w_j ; c_k = sqrt(clip(1 - sum w_j^2))
    y_k  = val / c_k ; q_k = (A_k - sum_j w_j q_j) / c_k
    h   += (-y_k q_k) @ Dn             (PE matmul accumulate into PSUM)
  backsolve L^T x = y, mu-law quantize x, recon = sum_k xq_k A_k,
  z_q = X + (recon - X), transpose back and DMA out.
"""

import math
from contextlib import ExitStack

import numpy as np

# ---- problem constants (hardcoded per spec) ----
N_CORES = 8
B_FULL, CD, Hh, Ww = 32, 64, 64, 64
HW = Hh * Ww                     # 4096
B_PER_CORE = B_FULL // N_CORES   # 4
NA = 256                         # atoms
K = 4                            # sparsity
TC = 8                           # chunks (of 128 signals) per block
BLK = TC * 128                   # 1024 signals per block
N_BLOCKS = B_PER_CORE * HW // BLK  # 16 blocks per core

MU = 50.0
CMAX = 3.0
NBINS = 16
EPS = 1e-10
COMMIT = 0.25
LOG1P_MU = float(np.log1p(50.0))          # float64, like reference
INV_LOG1P_MU = 1.0 / LOG1P_MU             # float64; f32-cast at use sites
F32 = None  # set lazily (mybir.dt.float32)

_NC_CACHE = {}


def _bcast_last(ap, n):
    """Append an innermost [step=0, count=n] dim to an AP (free-dim bcast)."""
    import concourse.bass as bass
    return bass.AP(tensor=ap.tensor, offset=ap.offset,
                   ap=[*[list(d) for d in ap.ap], [0, n]])


def build_nc(n_blocks=N_BLOCKS):
    import concourse.bass as bass
    import concourse.mybir as mybir
    import concourse.tile as tile
    from concourse import bacc
    from concourse.masks import make_identity

    f32 = mybir.dt.float32
    bf16 = mybir.dt.bfloat16
    ALU = mybir.AluOpType
    ACTF = mybir.ActivationFunctionType

    nc = bacc.Bacc(trn_type="TRN2", name="omp_vq")
    ze = nc.dram_tensor("ze", [B_PER_CORE, CD, HW], f32, kind="ExternalInput")
    dn = nc.dram_tensor("dn", [CD, NA], f32, kind="ExternalInput")
    dnt = nc.dram_tensor("dnt", [2, 128, CD], f32, kind="ExternalInput")
    zq = nc.dram_tensor("zq", [B_PER_CORE, CD, HW], f32, kind="ExternalOutput")

    dve = nc.vector
    act = nc.scalar
    pool_e = nc.gpsimd
    pe = nc.tensor

    def ts1(eng, out, in0, s1, op0):
        eng.tensor_scalar(out=out, in0=in0, scalar1=s1, scalar2=None, op0=op0)

    def ts2(eng, out, in0, s1, op0, s2, op1):
        eng.tensor_scalar(out=out, in0=in0, scalar1=s1, scalar2=s2,
                          op0=op0, op1=op1)

    with ExitStack() as ctx:
        tc = ctx.enter_context(tile.TileContext(nc))
        singles = ctx.enter_context(tc.tile_pool(name="singles", bufs=1))
        work = ctx.enter_context(tc.tile_pool(name="work", bufs=2))
        persist = ctx.enter_context(tc.tile_pool(name="persist", bufs=2))
        small = ctx.enter_context(tc.tile_pool(name="small", bufs=2))
        hpool = ctx.enter_context(tc.tile_pool(name="hps", bufs=1, space="PSUM"))
        trpool = ctx.enter_context(tc.tile_pool(name="trps", bufs=2, space="PSUM"))
        mmpool = ctx.enter_context(tc.tile_pool(name="mmps", bufs=2, space="PSUM"))

        # --- constants ---
        ident = singles.tile([128, 128], f32)
        make_identity(nc, ident)
        identb = singles.tile([128, 128], bf16)
        make_identity(nc, identb)
        dn_sb = singles.tile([CD, NA], f32)
        nc.sync.dma_start(out=dn_sb, in_=dn[:, :])
        dnt_sb = singles.tile([128, 2, CD], f32)
        for h in range(2):
            nc.sync.dma_start(out=dnt_sb[:, h, :], in_=dnt[h])

        # PE may carry at most one sem wait per instruction; consume each
        # constant's producer once so later matmuls never wait on them.
        warm = trpool.tile([128, 512], f32, tag="tr")
        pe.matmul(warm[0:1, 0:1], lhsT=dn_sb[:, 0:1], rhs=dn_sb[:, 0:1],
                  start=True, stop=True, skip_group_check=True)
        pe.matmul(warm[0:1, 1:2], lhsT=dnt_sb[:, 0, 0:1], rhs=dnt_sb[:, 0, 0:1],
                  start=True, stop=True, skip_group_check=True)
        pe.matmul(warm[0:1, 2:3], lhsT=dnt_sb[:, 1, 0:1], rhs=dnt_sb[:, 1, 0:1],
                  start=True, stop=True, skip_group_check=True)
        pe.matmul(warm[0:1, 3:4], lhsT=ident[:, 0:1], rhs=ident[:, 0:1],
                  start=True, stop=True, skip_group_check=True)

        for bi in range(n_blocks):
            b = bi // (HW // BLK)
            hw0 = (bi % (HW // BLK)) * BLK

            # ---------- setup: load X^T, h = X @ Dn, X signal-major ----------
            xt = work.tile([CD, BLK], f32, tag="xt")          # [64, 8*128]
            nc.sync.dma_start(out=xt, in_=ze[b, :, hw0:hw0 + BLK])

            # Chunks c and c+1 share a 2KB PSUM bank.  start=True marks the
            # WHOLE bank pending-zero, so only the even (bank-first) chunk may
            # use it; the odd chunk overwrites its still-pending half with
            # start=False.  Later psi-matmuls then accumulate (bits set).
            h_ps = hpool.tile([128, TC, NA], f32, tag="h")    # 4 banks
            for c in range(TC):
                pe.matmul(h_ps[:, c, :], lhsT=xt[:, c * 128:(c + 1) * 128],
                          rhs=dn_sb, start=(c % 2 == 0), stop=(c % 2 == 1),
                          skip_group_check=True)

            xs_ps = mmpool.tile([128, TC * CD], f32, tag="mm")  # 1 bank
            for c in range(TC):
                pe.transpose(xs_ps[:, c * CD:(c + 1) * CD],
                             xt[:, c * 128:(c + 1) * 128], ident[0:CD, 0:CD])
            x_sb = persist.tile([128, TC, CD], f32, tag="xsb")
            act.copy(x_sb, xs_ps.rearrange("p (t c) -> p t c", t=TC))

            # per-block per-signal state
            L_all = persist.tile([128, K, K, TC], f32, tag="Lall")
            y_all = persist.tile([128, K, TC], f32, tag="yall")
            a_all = persist.tile([128, K, TC, CD], f32, tag="Aall")
            q_sb = persist.tile([128, K - 1, TC, CD], f32, tag="qsb")

            for k in range(K):
                # ---------- selection ----------
                habs = work.tile([128, TC, NA], f32, tag="habs")
                act.activation(habs, h_ps, ACTF.Abs)
                vmax = small.tile([128, TC], f32, tag="vmax")
                dve.tensor_reduce(out=vmax, in_=habs, axis=mybir.AxisListType.X,
                                  op=ALU.max)
                e_t = work.tile([128, TC, NA], bf16, tag="E")
                for c in range(TC):
                    ts1(dve, e_t[:, c, :], habs[:, c, :], vmax[:, c:c + 1],
                        ALU.is_equal)

                # ---------- E^T via PE transposes (bf16, waves of 4) ----------
                et_sb = work.tile([128, 2 * TC, 128], f32, tag="etsb")
                for w in range(2 * TC // 4):
                    trb = trpool.tile([128, 512], bf16, tag="tr")
                    for i in range(4):
                        c = 2 * w + i // 2
                        hh = i % 2
                        pe.transpose(trb[:, i * 128:(i + 1) * 128],
                                     e_t[:, c, hh * 128:(hh + 1) * 128], identb)
                    act.copy(et_sb[:, 4 * w:4 * w + 4, :],
                             trb.rearrange("p (i q) -> p i q", i=4))

                # ---------- atom gather: A_k = E @ DnT ----------
                a_ps = mmpool.tile([128, TC * CD], f32, tag="mm")
                for c in range(TC):
                    for hh in range(2):
                        pe.matmul(a_ps[:, c * CD:(c + 1) * CD],
                                  lhsT=et_sb[:, 2 * c + hh, :],
                                  rhs=dnt_sb[:, hh, :],
                                  start=(hh == 0), stop=(hh == 1))
                a_k = a_all[:, k, :, :]
                act.copy(a_k, a_ps.rearrange("p (t c) -> p t c", t=TC))

                # ---------- dots: w_j = <q_j, A_k>, hb = <X, A_k> ----------
                hb = small.tile([128, TC], f32, tag="hb")
                for j in range(k + 1):
                    dotbuf = work.tile([128, TC, CD], f32, tag="dot")
                    opnd = q_sb[:, j, :, :] if j < k else x_sb
                    outt = L_all[:, k, j, :] if j < k else hb
                    eng = pool_e if j % 2 == 0 else dve
                    eng.tensor_tensor(out=dotbuf, in0=opnd, in1=a_k, op=ALU.mult)
                    dve.tensor_reduce(out=outt, in_=dotbuf,
                                      axis=mybir.AxisListType.X, op=ALU.add)

                # ---------- Cholesky scalars ----------
                if k == 0:
                    dve.memset(L_all[:, 0, 0, :], 1.0)
                    dve.tensor_copy(out=y_all[:, 0, :], in_=hb)
                else:
                    wsq = small.tile([128, TC], f32, tag="wsq")
                    yw = small.tile([128, TC], f32, tag="yw")
                    tmp8 = small.tile([128, TC], f32, tag="tmp8")
                    for j in range(k):
                        wj = L_all[:, k, j, :]
                        if j == 0:
                            dve.tensor_tensor(out=wsq, in0=wj, in1=wj, op=ALU.mult)
                            dve.tensor_tensor(out=yw, in0=y_all[:, 0, :], in1=wj,
                                              op=ALU.mult)
                        else:
                            dve.tensor_tensor(out=tmp8, in0=wj, in1=wj, op=ALU.mult)
                            dve.tensor_tensor(out=wsq, in0=wsq, in1=tmp8, op=ALU.add)
                            dve.tensor_tensor(out=tmp8, in0=y_all[:, j, :], in1=wj,
                                              op=ALU.mult)
                            dve.tensor_tensor(out=yw, in0=yw, in1=tmp8, op=ALU.add)
                    val8 = small.tile([128, TC], f32, tag="val8")
                    dve.tensor_tensor(out=val8, in0=hb, in1=yw, op=ALU.subtract)
                    onem = small.tile([128, TC], f32, tag="onem")
                    ts2(dve, onem, wsq, -1.0, ALU.mult, 1.0, ALU.add)
                    ts1(dve, onem, onem, 1e-12, ALU.max)
                    ck = L_all[:, k, k, :]
                    act.sqrt(ck, onem)
                    rcp8 = small.tile([128, TC], f32, tag="rcp8")
                    dve.reciprocal(out=rcp8, in_=ck)
                    dve.tensor_tensor(out=y_all[:, k, :], in0=val8, in1=rcp8,
                                      op=ALU.mult)

                # ---------- q_k and h update (not needed at k == K-1) ----------
                if k < K - 1:
                    qk = q_sb[:, k, :, :]
                    if k == 0:
                        # c_0 == 1 exactly; q_0 = A_0
                        dve.tensor_copy(out=qk, in_=a_k)
                    else:
                        accq = work.tile([128, TC, CD], f32, tag="accq")
                        src = a_k
                        for j in range(k):
                            tmpq = work.tile([128, TC, CD], f32, tag="tmpq")
                            pool_e.tensor_tensor(
                                out=tmpq, in0=q_sb[:, j, :, :],
                                in1=_bcast_last(L_all[:, k, j, :], CD), op=ALU.mult)
                            dve.tensor_tensor(out=accq, in0=src, in1=tmpq,
                                              op=ALU.subtract)
                            src = accq
                        dve.tensor_tensor(out=qk, in0=accq,
                                          in1=_bcast_last(rcp8, CD),
                                          op=ALU.mult)
                    # qt = q_k * (-y_k)
                    yneg = small.tile([128, TC], f32, tag="yneg")
                    ts1(dve, yneg, y_all[:, k, :], -1.0, ALU.mult)
                    qt = work.tile([128, TC, CD], f32, tag="qt")
                    dve.tensor_tensor(out=qt, in0=qk, in1=_bcast_last(yneg, CD),
                                      op=ALU.mult)
                    # transpose qt chunks -> [64, 128] and accumulate into h
                    qtt_sb = work.tile([CD, TC * 128], f32, tag="qtt")
                    for w in range(max(1, TC // 4)):
                        tr = trpool.tile([128, 512], f32, tag="tr")
                        for i in range(min(4, TC)):
                            c = 4 * w + i
                            pe.transpose(tr[0:CD, i * 128:(i + 1) * 128],
                                         qt[:, c, :], ident)
                        act.copy(qtt_sb[:, w * 512:(w + 1) * 512], tr[0:CD, :])
                    for c in range(TC):
                        pe.matmul(h_ps[:, c, :],
                                  lhsT=qtt_sb[:, c * 128:(c + 1) * 128],
                                  rhs=dn_sb, start=False, stop=True,
                                  skip_group_check=True)

            # ---------- backsolve L^T x = y ----------
            xsel = small.tile([128, TC, K], f32, tag="xsel")
            tmpb = small.tile([128, TC], f32, tag="tmpb")
            accb = small.tile([128, TC], f32, tag="accb")
            for m in range(K - 1, -1, -1):
                src = y_all[:, m, :]
                for j in range(m + 1, K):
                    dve.tensor_tensor(out=tmpb, in0=L_all[:, j, m, :],
                                      in1=xsel[:, :, j], op=ALU.mult)
                    dve.tensor_tensor(out=accb, in0=src, in1=tmpb, op=ALU.subtract)
                    src = accb
                rcpb = small.tile([128, TC], f32, tag="rcpb")
                dve.reciprocal(out=rcpb, in_=L_all[:, m, m, :])
                dve.tensor_tensor(out=xsel[:, :, m], in0=src,
                                  in1=rcpb, op=ALU.mult)

            # ---------- mu-law quantize (on [128, TC*K]) ----------
            xf = xsel.rearrange("p t k -> p (t k)")
            deq = small.tile([128, TC, K], f32, tag="deq")
            df = deq.rearrange("p t k -> p (t k)")
            t1 = small.tile([128, TC * K], f32, tag="qt1")
            t2 = small.tile([128, TC * K], f32, tag="qt2")
            t3 = small.tile([128, TC * K], f32, tag="qt3")
            ts2(dve, t1, xf, -CMAX, ALU.max, CMAX, ALU.min)    # clip
            ts1(dve, t1, t1, 1.0 / CMAX, ALU.mult)             # c = clip/3
            act.activation(t2, t1, ACTF.Abs)                   # |c|
            act.sign(t3, t1)                                   # sign(c)
            act.activation(t2, t2, ACTF.Ln, bias=1.0, scale=MU)  # ln(1+50|c|)
            dve.tensor_tensor(out=t2, in0=t3, in1=t2, op=ALU.mult)
            ts1(dve, t2, t2, INV_LOG1P_MU, ALU.mult)           # encoded
            ts2(dve, t2, t2, 1.0, ALU.add, (NBINS - 1) / 2.0, ALU.mult)
            ts1(dve, t2, t2, 8388608.0, ALU.add)               # round (RNE)
            ts1(dve, t2, t2, 8388608.0, ALU.subtract)
            ts2(dve, t2, t2, 0.0, ALU.max, float(NBINS - 1), ALU.min)  # bin
            ts2(dve, t2, t2, 2.0 / (NBINS - 1), ALU.mult, -1.0, ALU.add)  # z
            act.activation(t1, t2, ACTF.Abs)                   # |z|
            act.sign(t3, t2)                                   # sign(z)
            ts1(dve, t1, t1, LOG1P_MU, ALU.mult)
            act.activation(t1, t1, ACTF.Exp)
            ts1(dve, t1, t1, -1.0, ALU.add)                    # expm1
            ts1(dve, t1, t1, 1.0 / MU, ALU.mult)
            dve.tensor_tensor(out=t1, in0=t3, in1=t1, op=ALU.mult)
            ts1(dve, df, t1, CMAX, ALU.mult)                   # dequantized

            # ---------- recon, d, z_q ----------
            racc = work.tile([128, TC, CD], f32, tag="racc")
            rtmp = work.tile([128, TC, CD], f32, tag="rtmp")
            pool_e.tensor_tensor(out=racc, in0=a_all[:, 0, :, :],
                                 in1=_bcast_last(deq[:, :, 0], CD), op=ALU.mult)
            for m in range(1, K):
                eng = pool_e if m % 2 == 0 else dve
                eng.tensor_tensor(out=rtmp, in0=a_all[:, m, :, :],
                                  in1=_bcast_last(deq[:, :, m], CD), op=ALU.mult)
                dve.tensor_tensor(out=racc, in0=racc, in1=rtmp, op=ALU.add)
            dsb = work.tile([128, TC, CD], f32, tag="dsb")
            dve.tensor_tensor(out=dsb, in0=racc, in1=x_sb, op=ALU.subtract)
            zq_sb = work.tile([128, TC, CD], f32, tag="zqsb")
            pool_e.tensor_tensor(out=zq_sb, in0=x_sb, in1=dsb, op=ALU.add)

            # ---------- transpose z_q back and store ----------
            zqt_sb = work.tile([CD, BLK], f32, tag="zqt")
            for w in range(max(1, TC // 4)):
                tr = trpool.tile([128, 512], f32, tag="tr")
                for i in range(min(4, TC)):
                    c = 4 * w + i
                    pe.transpose(tr[0:CD, i * 128:(i + 1) * 128],
                                 zq_sb[:, c, :], ident)
                act.copy(zqt_sb[:, w * 512:(w + 1) * 512], tr[0:CD, :])
            nc.sync.dma_start(out=zq[b, :, hw0:hw0 + BLK], in_=zqt_sb)

    nc.compile()
    return nc


def _host_normalize(dictionary):
    """Normalize dictionary columns, matching reference numerics (f32)."""
    try:
        import jax
        cpu = jax.devices("cpu")[0]
        import jax.numpy as jnp
        with jax.default_device(cpu):
            d = jnp.asarray(dictionary, dtype=jnp.float32)
            nrm = jnp.linalg.norm(d, axis=0, keepdims=True)
            dn = d / jnp.maximum(nrm, EPS)
            return np.asarray(dn, dtype=np.float32)
    except Exception:
        d = dictionary.astype(np.float32)
        nrm = np.sqrt(np.sum(d * d, axis=0, keepdims=True, dtype=np.float32))
        return (d / np.maximum(nrm, np.float32(EPS))).astype(np.float32)


def _host_loss(z_q, z_e):
    """loss = q_latent + 0.25 * e_latent, replicating reference fp32 mean."""
    try:
        import jax
        cpu = jax.devices("cpu")[0]
        import jax.numpy as jnp
        with jax.default_device(cpu):
            d = jnp.asarray(z_q) - jnp.asarray(z_e)
            m = jnp.mean(d * d)
            loss = m + np.float32(COMMIT) * m
            return np.asarray(loss, dtype=np.float32)
    except Exception:
        d = (z_q - z_e).astype(np.float32)
        m = np.float32(np.mean(d.astype(np.float64) ** 2))
        return np.float32(m + np.float32(COMMIT) * m)


def get_nc():
    if "nc" not in _NC_CACHE:
        _NC_CACHE["nc"] = build_nc()
    return _NC_CACHE["nc"]


def _run(z_e, dictionary, **run_kwargs):
    from concourse.bass_utils import run_bass_kernel_spmd

    z_e = np.ascontiguousarray(np.asarray(z_e, dtype=np.float32))
    dictionary = np.ascontiguousarray(np.asarray(dictionary, dtype=np.float32))

    dn = _host_normalize(dictionary)                  # [64, 256]
    dnt = np.ascontiguousarray(dn.T).reshape(2, 128, CD).copy()

    ze_flat = z_e.reshape(B_FULL, CD, HW)
    in_maps = []
    for c in range(N_CORES):
        shard = np.ascontiguousarray(ze_flat[c * B_PER_CORE:(c + 1) * B_PER_CORE])
        in_maps.append({"ze": shard, "dn": dn, "dnt": dnt})

    nc = get_nc()
    res = run_bass_kernel_spmd(nc, in_maps, core_ids=list(range(N_CORES)),
                               **run_kwargs)
    zq_full = np.concatenate([r["zq"] for r in res.results], axis=0)
    zq_full = zq_full.reshape(B_FULL, CD, Hh, Ww)

    loss = _host_loss(zq_full, z_e)
    return zq_full, loss, res


def kernel(z_e, dictionary):
    zq_full, loss, _ = _run(z_e, dictionary)
    return zq_full, loss
